# revision 12
# baseline (speedup 1.0000x reference)
"""Trainium2 Bass kernel for CombinedGrAFFChemBERTa (gnn_message_passing).

Data-parallel over the batch axis: each of the 8 NeuronCores processes 128
molecules (contiguous 10-node / 9-edge blocks). Weights are replicated.
Cross-core coupling (GraphNorm edge stats per layer, global attention-pool
softmax) is handled with small AllReduce/AllGather collectives.

All matmuls run as float32r (PE rounds operands to reduced mantissa,
measured ~2e-4 rel err, full bf16-rate throughput). Activations are kept in
transposed layout x^T [H(part-chunks), nodes(free)] so every Linear maps to
lhsT=W, and gathers/scatters along the chain graph become contiguous
column-shifted views (edges padded to 10/molecule; pad column of the edge
features held at -1e30 so relu() kills it in the GINE aggregation).
"""

import os
import sys
import time

import numpy as np

for _p in ("/opt/trn_rl_repo", "/root/.axon_site/_ro/trn_rl_repo"):
    if _p not in sys.path and os.path.isdir(_p):
        sys.path.append(_p)

import concourse.bacc as bacc
import concourse.bass as bass
import concourse.mybir as mybir
import concourse.tile as tile
from concourse.masks import make_identity

F32 = mybir.dt.float32
F32R = mybir.dt.float32r
AX = mybir.AxisListType
ALU = mybir.AluOpType
ACTF = mybir.ActivationFunctionType

NCORES = 8
B, S, H = 1024, 128, 768
CB = B // NCORES            # molecules per core
NA, NB = 10, 9
NN = CB * NA                # nodes per core (1280)
DEPTH = int(os.environ.get("K_DEPTH", "6"))
KEIG, PHI = 8, 32
SUPP, FRAG = 256, 512
CH = H // 128               # 6 h-chunks
MZ_MAX, PROB_THR = 2000.0, 1e-4
E_GLOBAL = B * NB           # 9216 edges globally (graphnorm denominator)

# column chunks over the (padded) node/edge axis
NCHUNKS = [(0, 480), (480, 960), (960, 1280)]

_RUNNER = None


# --------------------------------------------------------------------------
# program builder
# --------------------------------------------------------------------------

def _build_nc():
    nc = bacc.Bacc("TRN2", target_bir_lowering=False, debug=False,
                   num_devices=NCORES)

    def din(name, shape, dt=F32R):
        return nc.dram_tensor(name, list(shape), dt, kind="ExternalInput")

    T = {}
    T["tok"] = din("tok", [CB, S, H])
    T["aggC"] = din("aggC", [S, 20])
    T["u4a"] = din("u4a", [KEIG, 2 * NN])
    T["u4b"] = din("u4b", [KEIG, 2 * NN])
    T["phiW1bd"] = din("phiW1bd", [KEIG, 128])
    T["phiW2bd"] = din("phiW2bd", [128, 128])
    T["phib1r"] = din("phib1r", [128], F32)
    T["phib2r"] = din("phib2r", [128], F32)
    T["rhoW1"] = din("rhoW1", [2 * 128, H])
    T["rhob1"] = din("rhob1", [H], F32)
    T["rhoW2"] = din("rhoW2", [H, H])
    T["rhob2"] = din("rhob2", [H], F32)
    T["gw1"] = din("gw1", [DEPTH, H, H])
    T["gb1"] = din("gb1", [DEPTH, H], F32)
    T["gw2"] = din("gw2", [DEPTH, H, H])
    T["gb2"] = din("gb2", [DEPTH, H], F32)
    T["geps"] = din("geps", [DEPTH], F32)
    T["ew1"] = din("ew1", [DEPTH, 3 * H, H])
    T["eb1"] = din("eb1", [DEPTH, H], F32)
    T["ew2"] = din("ew2", [DEPTH, H, H])
    T["eb2"] = din("eb2", [DEPTH, H], F32)
    T["gnw"] = din("gnw", [DEPTH, H], F32)
    T["gnb"] = din("gnb", [DEPTH, H], F32)
    T["gna"] = din("gna", [DEPTH, H], F32)
    T["attnW"] = din("attnW", [H, 1])
    T["supT"] = din("supT", [SUPP, CB])
    T["supW"] = din("supW", [SUPP, H])
    T["supb"] = din("supb", [1, H])
    T["suplnw"] = din("suplnw", [H], F32)
    T["suplnb"] = din("suplnb", [H], F32)
    T["fc1W"] = din("fc1W", [H, H])
    T["fc1b"] = din("fc1b", [1, H])
    T["fc2W"] = din("fc2W", [H, H])
    T["fc2b"] = din("fc2b", [1, H])
    T["mlplnw"] = din("mlplnw", [H], F32)
    T["mlplnb"] = din("mlplnb", [H], F32)
    T["mzW"] = din("mzW", [H, FRAG])
    T["mzb"] = din("mzb", [1, FRAG])
    T["probW"] = din("probW", [H, FRAG])
    T["probb"] = din("probb", [1, FRAG])
    T["ones1"] = din("ones1", [1, 128])
    T["out"] = nc.dram_tensor("out", [CB, FRAG, 2], F32, kind="ExternalOutput")

    with tile.TileContext(nc) as tc:
        _emit(nc, tc, T)
    nc.compile()
    return nc


def _bcast_row(nc, pool, dram_t, n, name):
    """[n] dram vector -> [128, n] sbuf tile broadcast across partitions."""
    t = pool.tile([128, n], F32, name=name)
    src = bass.AP(tensor=dram_t.ap().tensor, offset=0, ap=[[0, 128], [1, n]])
    nc.sync.dma_start(out=t[:], in_=src)
    return t


def _part_col(nc, pool, dram_ap_1d, n, name, tag=""):
    """[128*n] dram vector -> [128, n] tile, partition-major chunks."""
    t = pool.tile([128, n], F32, name=name, tag=tag or name)
    nc.sync.dma_start(out=t[:], in_=dram_ap_1d.rearrange("(c p) -> p c", p=128))
    return t


def _emit(nc, tc, T):
    out = T["out"]

    consts = tc.alloc_tile_pool(name="consts", bufs=1)
    state = tc.alloc_tile_pool(name="state", bufs=1)
    state_e = tc.alloc_tile_pool(name="state_e", bufs=1)

    ident = consts.tile([128, 128], F32)
    make_identity(nc, ident[:])
    eps1 = consts.tile([128, DEPTH], F32)   # 1 + gine_eps
    src = bass.AP(tensor=T["geps"].ap().tensor, offset=0,
                  ap=[[0, 128], [1, DEPTH]])
    nc.sync.dma_start(out=eps1[:], in_=src)
    nc.scalar.add(out=eps1[:], in_=eps1[:], add=1.0)
    eps5 = consts.tile([128, 1], F32)
    nc.vector.memset(eps5[:], 1e-5)
    zeroC = consts.tile([128, 128], F32)
    nc.vector.memset(zeroC[:], 0.0)
    negC = consts.tile([128, 128], F32)
    nc.vector.memset(negC[:], -(2.0 ** 100))

    # master activations (transposed layout)
    xT = state.tile([128, CH, NN + 1], F32R)     # +1 scratch col for dst shift
    eT = state_e.tile([128, CH, NN], F32R)       # padded edges, pad col -1e30
    nc.vector.tensor_copy(out=xT[:, :, NN:NN + 1],
                          in_=zeroC[:, 0:CH].rearrange("p (a b) -> p a b", b=1))
    for c in range(CH):
        nc.vector.tensor_copy(
            out=eT[:, c, :].rearrange("p (m a) -> p m a", a=NA)[:, :, NB:NA],
            in_=negC[:, 0:CB].rearrange("p (m b) -> p m b", b=1))

    # ---------------- phase 0: token aggregation + SignNet ----------------
    with tc.tile_pool(name="p0", bufs=1) as p0, \
         tc.tile_pool(name="p0s", bufs=2) as p0s, \
         tc.tile_pool(name="p0tok", bufs=4) as p0tok, \
         tc.tile_pool(name="ps_agg", bufs=6, space="PSUM") as ps_agg, \
         tc.tile_pool(name="ps_phi", bufs=2, space="PSUM") as ps_phi:

        aggC_sb = p0.tile([128, 20], F32R)
        nc.sync.dma_start(out=aggC_sb[:], in_=T["aggC"].ap())

        # --- SignNet phi (overlaps token aggregation on other engines) ---
        phiW1_sb = p0.tile([KEIG, 128], F32R)
        nc.sync.dma_start(out=phiW1_sb[:], in_=T["phiW1bd"].ap())
        phiW2_sb = p0.tile([128, 128], F32R)
        nc.sync.dma_start(out=phiW2_sb[:], in_=T["phiW2bd"].ap())
        phib1_sb = p0.tile([128, 1], F32)
        nc.sync.dma_start(out=phib1_sb[:],
                          in_=bass.AP(tensor=T["phib1r"].ap().tensor, offset=0,
                                      ap=[[1, 128], [1, 1]]))
        phib2_sb = p0.tile([128, 1], F32)
        nc.sync.dma_start(out=phib2_sb[:],
                          in_=bass.AP(tensor=T["phib2r"].ap().tensor, offset=0,
                                      ap=[[1, 128], [1, 1]]))
        rhoW1_sb = p0.tile([128, 2, H], F32R)
        nc.sync.dma_start(out=rhoW1_sb[:],
                          in_=T["rhoW1"].ap().rearrange("(k p) m -> p k m", p=128))
        rhoW2_sb = p0.tile([128, CH, H], F32R)
        nc.sync.dma_start(out=rhoW2_sb[:],
                          in_=T["rhoW2"].ap().rearrange("(k p) m -> p k m", p=128))
        rhob1_sb = _part_col(nc, p0, T["rhob1"].ap(), CH, "rhob1_sb")
        rhob2_sb = _part_col(nc, p0, T["rhob2"].ap(), CH, "rhob2_sb")

        rho_in = []
        for gi, u in enumerate((T["u4a"], T["u4b"])):
            u_sb = p0.tile([KEIG, 2 * NN], F32R, name=f"u_sb{gi}")
            nc.sync.dma_start(out=u_sb[:], in_=u.ap())
            h1 = p0.tile([128, 2 * NN], F32R, name=f"h1_{gi}")
            for c0 in range(0, 2 * NN, 512):
                w = min(512, 2 * NN - c0)
                ps = ps_phi.tile([128, 512], F32, tag="phips", name="psp1")
                nc.tensor.matmul(ps[:, :w], phiW1_sb[:], u_sb[:, c0:c0 + w],
                                 start=True, stop=True)
                nc.scalar.activation(out=h1[:, c0:c0 + w], in_=ps[:, :w],
                                     func=ACTF.Relu, bias=phib1_sb[:])
            h2 = p0.tile([128, 2 * NN], F32, name=f"h2_{gi}")
            for c0 in range(0, 2 * NN, 512):
                w = min(512, 2 * NN - c0)
                ps = ps_phi.tile([128, 512], F32, tag="phips", name="psp2")
                nc.tensor.matmul(ps[:, :w], phiW2_sb[:], h1[:, c0:c0 + w],
                                 start=True, stop=True)
                nc.scalar.activation(out=h2[:, c0:c0 + w], in_=ps[:, :w],
                                     func=ACTF.Identity, bias=phib2_sb[:])
            ri = p0.tile([128, NN], F32R, name=f"rho_in{gi}")
            nc.vector.tensor_add(out=ri[:], in0=h2[:, :NN], in1=h2[:, NN:])
            rho_in.append(ri)

        # r1 = relu(rho_in @ rhoW1 + b1)
        r1 = p0.tile([128, CH, NN], F32R)
        for (n0, n1) in NCHUNKS:
            w = n1 - n0
            for m in range(CH):
                ps = ps_phi.tile([128, 512], F32, tag="phips", name="psr1")
                for k in range(2):
                    nc.tensor.matmul(ps[:, :w],
                                     rhoW1_sb[:, k, m * 128:(m + 1) * 128],
                                     rho_in[k][:, n0:n1],
                                     start=(k == 0), stop=(k == 1))
                nc.scalar.activation(out=r1[:, m, n0:n1], in_=ps[:, :w],
                                     func=ACTF.Relu, bias=rhob1_sb[:, m:m + 1])

        # --- token aggregation: per-molecule lhsT=tokens, rhs=C ---
        GRP = 25
        for g0 in range(0, CB, GRP):
            gmols = min(GRP, CB - g0)
            pss = [ps_agg.tile([128, GRP * 20], F32, tag="aggps",
                               name=f"aggps{c}") for c in range(CH)]
            for j in range(gmols):
                t_sb = p0tok.tile([128, H], F32R, tag="tok", name="tok_sb")
                nc.sync.dma_start(out=t_sb[:], in_=T["tok"].ap()[g0 + j])
                for c in range(CH):
                    o = j * 20
                    nc.tensor.matmul(pss[c][:, o:o + 20],
                                     t_sb[:, c * 128:(c + 1) * 128],
                                     aggC_sb[:], start=True, stop=True)
            for c in range(CH):
                v = pss[c][:, :gmols * 20].rearrange(
                    "p (m a) -> p m a", a=20)
                nc.vector.tensor_copy(
                    out=xT[:, c, g0 * NA:(g0 + gmols) * NA].rearrange(
                        "p (m a) -> p m a", a=NA),
                    in_=v[:, :, 0:NA])
                nc.vector.tensor_copy(
                    out=eT[:, c, g0 * NA:(g0 + gmols) * NA].rearrange(
                        "p (m a) -> p m a", a=NA)[:, :, 0:NB],
                    in_=v[:, :, NA:NA + NB])

        # x = node_feat + x_eig (rho2 added straight onto xT)
        for (n0, n1) in NCHUNKS:
            w = n1 - n0
            for m in range(CH):
                ps = ps_phi.tile([128, 512], F32, tag="phips", name="psr2")
                for k in range(CH):
                    nc.tensor.matmul(ps[:, :w],
                                     rhoW2_sb[:, k, m * 128:(m + 1) * 128],
                                     r1[:, k, n0:n1],
                                     start=(k == 0), stop=(k == CH - 1))
                scr0 = p0s.tile([128, 512], F32, tag="scr0", name="scr0")
                nc.scalar.activation(out=scr0[:, :w], in_=ps[:, :w],
                                     func=ACTF.Identity, bias=rhob2_sb[:, m:m + 1])
                nc.vector.tensor_add(out=xT[:, m, n0:n1], in0=xT[:, m, n0:n1],
                                     in1=scr0[:, :w])

    # ---------------- GINE layers ----------------
    with tc.tile_pool(name="lw", bufs=1) as lw, \
         tc.tile_pool(name="lb", bufs=2) as lb, \
         tc.tile_pool(name="lscr", bufs=1) as lscr, \
         tc.tile_pool(name="lt1", bufs=1) as lt1, \
         tc.tile_pool(name="le1", bufs=1) as le1, \
         tc.tile_pool(name="ldram", bufs=1, space="DRAM") as ldram, \
         tc.tile_pool(name="ps_l", bufs=6, space="PSUM") as ps_l:

        def load_w(dram2d, kc, tag):
            t = lw.tile([128, kc, H], F32R, tag=tag, name=tag)
            src = dram2d.rearrange("(k p) m -> p k m", p=128)
            for k in range(kc):
                nc.sync.dma_start(out=t[:, k, :], in_=src[:, k, :])
            return t

        for l in range(DEPTH):
            b1 = _part_col(nc, lb, T["gb1"].ap()[l], CH, f"b1_{l}", "b1")
            b2 = _part_col(nc, lb, T["gb2"].ap()[l], CH, f"b2_{l}", "b2")
            e_b1 = _part_col(nc, lb, T["eb1"].ap()[l], CH, f"eb1_{l}", "eb1")
            e_b2 = _part_col(nc, lb, T["eb2"].ap()[l], CH, f"eb2_{l}", "eb2")
            gn_w = _part_col(nc, lb, T["gnw"].ap()[l], CH, f"gnw_{l}", "gnw")
            gn_b = _part_col(nc, lb, T["gnb"].ap()[l], CH, f"gnb_{l}", "gnb")
            gn_a = _part_col(nc, lb, T["gna"].ap()[l], CH, f"gna_{l}", "gna")

            # ---- hm (in place on xT): x = (1+eps)x + scatter(relu(x+e)) ----
            for c in range(CH):
                scr = lscr.tile([128, NN], F32, tag="scr", name=f"rl{l}_{c}")
                nc.vector.tensor_add(out=scr[:], in0=xT[:, c, :NN],
                                     in1=eT[:, c, :])
                nc.scalar.activation(out=scr[:], in_=scr[:], func=ACTF.Relu)
                nc.scalar.activation(out=xT[:, c, :NN], in_=xT[:, c, :NN],
                                     func=ACTF.Identity, scale=eps1[:, l:l + 1])
                nc.vector.tensor_add(out=xT[:, c, 1:NN], in0=xT[:, c, 1:NN],
                                     in1=scr[:, :NN - 1])

            # ---- node MLP: x = hm + silu(hm@W1+b1)@W2 + b2  (hm == xT) ----
            w1 = load_w(T["gw1"].ap()[l], CH, "w1")
            w2 = load_w(T["gw2"].ap()[l], CH, "w2")
            for (n0, n1) in NCHUNKS:
                w = n1 - n0
                t1 = lt1.tile([128, CH, 480], F32R, tag="t1", name=f"t1_{l}")
                for m in range(CH):
                    ps = ps_l.tile([128, 480], F32, tag="ps", name=f"pA{l}{m}")
                    for k in range(CH):
                        nc.tensor.matmul(ps[:, :w],
                                         w1[:, k, m * 128:(m + 1) * 128],
                                         xT[:, k, n0:n1],
                                         start=(k == 0), stop=(k == CH - 1))
                    nc.scalar.activation(out=t1[:, m, :w], in_=ps[:, :w],
                                         func=ACTF.Silu, bias=b1[:, m:m + 1])
                for m in range(CH):
                    ps = ps_l.tile([128, 480], F32, tag="ps", name=f"pB{l}{m}")
                    for k in range(CH):
                        nc.tensor.matmul(ps[:, :w],
                                         w2[:, k, m * 128:(m + 1) * 128],
                                         t1[:, k, :w],
                                         start=(k == 0), stop=(k == CH - 1))
                    scr2 = lscr.tile([128, 480], F32, tag="scr2",
                                     name=f"xu{l}{m}")
                    nc.scalar.activation(out=scr2[:, :w], in_=ps[:, :w],
                                         func=ACTF.Identity, bias=b2[:, m:m + 1])
                    nc.vector.tensor_add(out=xT[:, m, n0:n1],
                                         in0=xT[:, m, n0:n1], in1=scr2[:, :w])

            # ---- edge MLP: e = silu([e,x_src,x_dst]@eW1+b1)@eW2+b2 ----
            ew1 = load_w(T["ew1"].ap()[l], 3 * CH, "ew1")
            ew2 = load_w(T["ew2"].ap()[l], CH, "ew2")
            for (n0, n1) in NCHUNKS:
                w = n1 - n0
                e1 = le1.tile([128, CH, 480], F32R, tag="e1", name=f"e1_{l}")
                for m in range(CH):
                    ps = ps_l.tile([128, 480], F32, tag="ps", name=f"pE{l}{m}")
                    for k in range(3 * CH):
                        if k < CH:
                            rhs = eT[:, k, n0:n1]
                        elif k < 2 * CH:
                            rhs = xT[:, k - CH, n0:n1]
                        else:
                            rhs = xT[:, k - 2 * CH, n0 + 1:n1 + 1]
                        nc.tensor.matmul(ps[:, :w],
                                         ew1[:, k, m * 128:(m + 1) * 128],
                                         rhs, start=(k == 0),
                                         stop=(k == 3 * CH - 1))
                    nc.scalar.activation(out=e1[:, m, :w], in_=ps[:, :w],
                                         func=ACTF.Silu, bias=e_b1[:, m:m + 1])
                for m in range(CH):
                    ps = ps_l.tile([128, 480], F32, tag="ps", name=f"pF{l}{m}")
                    for k in range(CH):
                        nc.tensor.matmul(ps[:, :w],
                                         ew2[:, k, m * 128:(m + 1) * 128],
                                         e1[:, k, :w],
                                         start=(k == 0), stop=(k == CH - 1))
                    nc.scalar.activation(out=eT[:, m, n0:n1], in_=ps[:, :w],
                                         func=ACTF.Identity, bias=e_b2[:, m:m + 1])

            # ---- GraphNorm over all real edges (cross-core AllReduce) ----
            stats = lb.tile([128, CH, 2], F32, tag="st", name=f"st_{l}")
            for c in range(CH):
                v = eT[:, c, :].rearrange("p (m a) -> p m a", a=NA)[:, :, 0:NB]
                nc.vector.tensor_reduce(out=stats[:, c, 0:1], in_=v,
                                        axis=AX.XY, op=ALU.add)
                sq = lscr.tile([128, NN], F32, tag="scr", name=f"sq{l}_{c}")
                nc.scalar.activation(
                    out=sq[:, :CB * NB].rearrange("p (m a) -> p m a", a=NB),
                    in_=v, func=ACTF.Square, accum_out=stats[:, c, 1:2])
            gin = ldram.tile([128, CH * 2], F32, name=f"gin_{l}")
            gout = ldram.tile([128, CH * 2], F32, name=f"gout_{l}")
            nc.sync.dma_start(out=gin[:],
                              in_=stats.rearrange("p a b -> p (a b)"))
            nc.gpsimd.collective_compute(
                "AllReduce", ALU.add, replica_groups=[list(range(NCORES))],
                ins=[gin.opt()], outs=[gout.opt()])
            gst = lb.tile([128, CH, 2], F32, tag="gst", name=f"gst_{l}")
            nc.sync.dma_start(out=gst.rearrange("p a b -> p (a b)"),
                              in_=gout[:])

            # mu=S1/E; m2=S2/E; var=m2-(2a-a^2)mu^2; s=gnw*rsqrt(var+1e-5)
            # shift=gnb-a*mu*s;   e = e*s + shift
            mu = lb.tile([128, CH], F32, tag="mu", name=f"mu_{l}")
            var = lb.tile([128, CH], F32, tag="var", name=f"var_{l}")
            ca = lb.tile([128, CH], F32, tag="ca", name=f"ca_{l}")
            s_ = lb.tile([128, CH], F32, tag="s_", name=f"s_{l}")
            sh = lb.tile([128, CH], F32, tag="sh", name=f"sh_{l}")
            nc.scalar.mul(out=mu[:], in_=gst[:, :, 0], mul=1.0 / E_GLOBAL)
            nc.scalar.mul(out=var[:], in_=gst[:, :, 1], mul=1.0 / E_GLOBAL)
            # ca = (2a - a^2) * mu^2
            nc.vector.tensor_mul(out=ca[:], in0=gn_a[:], in1=gn_a[:])
            nc.vector.scalar_tensor_tensor(out=ca[:], in0=gn_a[:], scalar=2.0,
                                           in1=ca[:], op0=ALU.mult,
                                           op1=ALU.subtract)
            nc.vector.tensor_mul(out=ca[:], in0=ca[:], in1=mu[:])
            nc.vector.tensor_mul(out=ca[:], in0=ca[:], in1=mu[:])
            nc.vector.tensor_sub(out=var[:], in0=var[:], in1=ca[:])
            nc.scalar.activation(out=s_[:], in_=var[:], func=ACTF.Sqrt,
                                 bias=eps5[:])
            nc.vector.reciprocal(out=s_[:], in_=s_[:])
            nc.vector.tensor_mul(out=s_[:], in0=s_[:], in1=gn_w[:])
            nc.vector.tensor_mul(out=sh[:], in0=mu[:], in1=gn_a[:])
            nc.vector.tensor_mul(out=sh[:], in0=sh[:], in1=s_[:])
            nc.vector.tensor_sub(out=sh[:], in0=gn_b[:], in1=sh[:])
            for c in range(CH):
                nc.scalar.activation(out=eT[:, c, :], in_=eT[:, c, :],
                                     func=ACTF.Identity, scale=s_[:, c:c + 1],
                                     bias=sh[:, c:c + 1])
                nc.vector.tensor_copy(
                    out=eT[:, c, :].rearrange("p (m a) -> p m a",
                                              a=NA)[:, :, NB:NA],
                    in_=negC[:, 0:CB].rearrange("p (m b) -> p m b", b=1))

    state_e.release()

    # ---------------- attention pool + head ----------------
    with tc.tile_pool(name="tl", bufs=1) as tl, \
         tc.tile_pool(name="tw", bufs=1) as tw, \
         tc.tile_pool(name="tdram", bufs=1, space="DRAM") as tdram, \
         tc.tile_pool(name="ps_t", bufs=2, space="PSUM") as ps_t:

        suplnw_b = _bcast_row(nc, tw, T["suplnw"], H, "suplnw_b")
        suplnb_b = _bcast_row(nc, tw, T["suplnb"], H, "suplnb_b")
        mlplnw_b = _bcast_row(nc, tw, T["mlplnw"], H, "mlplnw_b")
        mlplnb_b = _bcast_row(nc, tw, T["mlplnb"], H, "mlplnb_b")

        attnW_sb = tw.tile([128, CH, 128], F32R)
        for c in range(CH):
            nc.vector.tensor_copy(out=attnW_sb[:, c, :], in_=zeroC[:])
            nc.sync.dma_start(
                out=attnW_sb[:, c, 0:1],
                in_=T["attnW"].ap()[c * 128:(c + 1) * 128, :])
        sc = tl.tile([1, NN], F32)
        for (n0, n1) in NCHUNKS:
            w = n1 - n0
            ps = ps_t.tile([128, 512], F32, tag="pssc", name="pssc")
            for k in range(CH):
                nc.tensor.matmul(ps[:, :w], attnW_sb[:, k, :],
                                 xT[:, k, n0:n1], start=(k == 0),
                                 stop=(k == CH - 1))
            nc.vector.tensor_copy(out=sc[:, n0:n1], in_=ps[0:1, :w])
        mx = tl.tile([1, 1], F32)
        nmx = tl.tile([1, 1], F32)
        nc.vector.tensor_reduce(out=mx[:], in_=sc[:], axis=AX.X, op=ALU.max)
        nc.scalar.mul(out=nmx[:], in_=mx[:], mul=-1.0)
        wexp = tl.tile([1, NN], F32R)
        ssum = tl.tile([1, 1], F32)
        nc.scalar.activation(out=wexp[:], in_=sc[:], func=ACTF.Exp,
                             bias=nmx[:], accum_out=ssum[:])

        # cross-core softmax stats
        stg_in = tdram.tile([1, 2], F32)
        stg_out = tdram.tile([NCORES, 2], F32)
        stl = tl.tile([1, 2], F32)
        nc.vector.tensor_copy(out=stl[:, 0:1], in_=mx[:])
        nc.vector.tensor_copy(out=stl[:, 1:2], in_=ssum[:])
        nc.sync.dma_start(out=stg_in[:], in_=stl[:])
        nc.gpsimd.collective_compute(
            "AllGather", ALU.bypass, replica_groups=[list(range(NCORES))],
            ins=[stg_in.opt()], outs=[stg_out.opt()])
        stg = tl.tile([1, NCORES, 2], F32)
        nc.sync.dma_start(
            out=stg.rearrange("p a b -> p (a b)"),
            in_=bass.AP(tensor=stg_out.tensor, offset=stg_out.offset,
                        ap=[[0, 1], [1, 2 * NCORES]]))

        mg = tl.tile([1, 1], F32)
        nmg = tl.tile([1, 1], F32)
        nc.vector.tensor_reduce(out=mg[:], in_=stg[:, :, 0], axis=AX.X,
                                op=ALU.max)
        nc.scalar.mul(out=nmg[:], in_=mg[:], mul=-1.0)
        zex = tl.tile([1, NCORES], F32)
        nc.scalar.activation(out=zex[:], in_=stg[:, :, 0], func=ACTF.Exp,
                             bias=nmg[:])
        nc.vector.tensor_mul(out=zex[:], in0=zex[:], in1=stg[:, :, 1])
        zg = tl.tile([1, 1], F32)
        nc.vector.tensor_reduce(out=zg[:], in_=zex[:], axis=AX.X, op=ALU.add)
        nc.vector.reciprocal(out=zg[:], in_=zg[:])
        scal = tl.tile([1, 1], F32)
        nc.scalar.activation(out=scal[:], in_=mx[:], func=ACTF.Exp,
                             bias=nmg[:])
        nc.vector.tensor_mul(out=scal[:], in0=scal[:], in1=zg[:])
        scal_d = tdram.tile([1, 1], F32)
        nc.sync.dma_start(out=scal_d[:], in_=scal[:])
        scal_b = tl.tile([128, 1], F32)
        nc.sync.dma_start(out=scal_b[:],
                          in_=bass.AP(tensor=scal_d.tensor, offset=scal_d.offset,
                                      ap=[[0, 128], [1, 1]]))

        ones_sb = tw.tile([1, 128], F32R)
        nc.sync.dma_start(out=ones_sb[:], in_=T["ones1"].ap())
        wb = tl.tile([128, NN], F32)
        for (n0, n1) in NCHUNKS:
            w = n1 - n0
            psb = ps_t.tile([128, 512], F32, tag="psb", name="psb")
            nc.tensor.matmul(psb[:, :w], ones_sb[:], wexp[:, n0:n1],
                             start=True, stop=True)
            nc.scalar.copy(out=wb[:, n0:n1], in_=psb[:, :w])

        # numer^T[c, mol] = sum over atoms of x*w
        numT = tl.tile([128, CH, CB], F32)
        for c in range(CH):
            xw = tl.tile([128, NN], F32, tag="xw", name=f"xw{c}")
            nc.vector.tensor_mul(out=xw[:], in0=xT[:, c, :NN], in1=wb[:])
            nc.vector.tensor_reduce(
                out=numT[:, c, :],
                in_=xw.rearrange("p (m a) -> p m a", a=NA),
                axis=AX.X, op=ALU.add)

        # ---- supp path (natural layout) ----
        supT_sb = tw.tile([128, 2, CB], F32R)
        nc.sync.dma_start(out=supT_sb[:],
                          in_=T["supT"].ap().rearrange("(k p) m -> p k m", p=128))
        supW_sb = tw.tile([128, 2, H], F32R)
        nc.sync.dma_start(out=supW_sb[:],
                          in_=T["supW"].ap().rearrange("(k p) m -> p k m", p=128))
        supb_sb = tw.tile([1, H], F32R)
        nc.sync.dma_start(out=supb_sb[:], in_=T["supb"].ap())

        def nat_matmul(lhsT_of_k, rhs_sb, bias_sb, n_out, kc, psname):
            pss = []
            for h0 in range(0, n_out, 384):
                w = min(384, n_out - h0)
                ps = ps_t.tile([128, 384], F32, tag="psn", name=f"{psname}{h0}")
                for k in range(kc):
                    nc.tensor.matmul(ps[:, :w], lhsT_of_k(k),
                                     rhs_sb[:, k, h0:h0 + w],
                                     start=(k == 0), stop=False)
                nc.tensor.matmul(ps[:, :w], ones_sb[:], bias_sb[:, h0:h0 + w],
                                 start=False, stop=True)
                pss.append((ps, h0, w))
            return pss

        _ln_n = [0]

        def ln_apply(dst, pss, lnw_b, lnb_b, gelu):
            """LayerNorm over the (free-dim) pieces in pss + affine -> dst."""
            _ln_n[0] += 1
            u = _ln_n[0]
            ngr = sum(w // 128 for _, _, w in pss)
            stt = tl.tile([128, ngr, nc.vector.BN_STATS_DIM], F32,
                          tag="bst", name=f"bst{u}")
            i = 0
            for ps, h0, w in pss:
                for s0 in range(0, w, 128):
                    nc.vector.bn_stats(out=stt[:, i, :],
                                       in_=ps[:, s0:s0 + 128])
                    i += 1
            mv = tl.tile([128, nc.vector.BN_AGGR_DIM], F32, tag="bmv",
                         name=f"bmv{u}")
            nc.vector.bn_aggr(out=mv[:], in_=stt[:])
            rstd = tl.tile([128, 1], F32, tag="rstd", name=f"rs{u}")
            nc.scalar.activation(out=rstd[:], in_=mv[:, 1:2], func=ACTF.Sqrt,
                                 bias=eps5[:])
            nc.vector.reciprocal(out=rstd[:], in_=rstd[:])
            for ps, h0, w in pss:
                nc.vector.tensor_scalar(out=dst[:, h0:h0 + w], in0=ps[:, :w],
                                        scalar1=mv[:, 0:1], scalar2=rstd[:],
                                        op0=ALU.subtract, op1=ALU.mult)
            nc.vector.tensor_mul(out=dst[:], in0=dst[:], in1=lnw_b[:])
            nc.vector.tensor_add(out=dst[:], in0=dst[:], in1=lnb_b[:])
            if gelu:
                nc.scalar.activation(out=dst[:], in_=dst[:], func=ACTF.Gelu)

        supp_g = tl.tile([128, H], F32)
        pss = nat_matmul(lambda k: supT_sb[:, k, :], supW_sb, supb_sb, H, 2,
                         "sup")
        ln_apply(supp_g, pss, suplnw_b, suplnb_b, gelu=True)

        def transpose6(src_nat, dst_T):
            for c in range(CH):
                pst = ps_t.tile([128, 128], F32, tag="pstr", name="pstr")
                nc.tensor.transpose(pst[:], src_nat[:, c * 128:(c + 1) * 128],
                                    ident[:])
                nc.scalar.copy(out=dst_T[:, c, :], in_=pst[:])

        suppgT = tl.tile([128, CH, CB], F32)
        transpose6(supp_g, suppgT)
        combT = tl.tile([128, CH, CB], F32R)
        for c in range(CH):
            nc.vector.tensor_scalar(out=combT[:, c, :], in0=numT[:, c, :],
                                    scalar1=scal_b[:], scalar2=None,
                                    op0=ALU.mult)
            nc.vector.tensor_add(out=combT[:, c, :], in0=combT[:, c, :],
                                 in1=suppgT[:, c, :])
        comb_nat = tl.tile([128, H], F32)
        for c in range(CH):
            pst = ps_t.tile([128, 128], F32, tag="pstr", name="pstr2")
            nc.tensor.transpose(pst[:], combT[:, c, :].bitcast(F32), ident[:])
            nc.scalar.copy(out=comb_nat[:, c * 128:(c + 1) * 128], in_=pst[:])

        # fc1 -> gelu -> fc2 -> +residual -> LN
        fc1W_sb = tw.tile([128, CH, H], F32R)
        nc.sync.dma_start(out=fc1W_sb[:],
                          in_=T["fc1W"].ap().rearrange("(k p) m -> p k m", p=128))
        fc1b_sb = tw.tile([1, H], F32R)
        nc.sync.dma_start(out=fc1b_sb[:], in_=T["fc1b"].ap())
        g1 = tl.tile([128, H], F32)
        pss = nat_matmul(lambda k: combT[:, k, :], fc1W_sb, fc1b_sb, H, CH,
                         "fc1")
        for ps, h0, w in pss:
            nc.scalar.activation(out=g1[:, h0:h0 + w], in_=ps[:, :w],
                                 func=ACTF.Gelu)
        g1T = tl.tile([128, CH, CB], F32R)
        transpose6(g1, g1T)
        fc2W_sb = tw.tile([128, CH, H], F32R)
        nc.sync.dma_start(out=fc2W_sb[:],
                          in_=T["fc2W"].ap().rearrange("(k p) m -> p k m", p=128))
        fc2b_sb = tw.tile([1, H], F32R)
        nc.sync.dma_start(out=fc2b_sb[:], in_=T["fc2b"].ap())
        ypre = tl.tile([128, H], F32)
        pss = nat_matmul(lambda k: g1T[:, k, :], fc2W_sb, fc2b_sb, H, CH,
                         "fc2")
        for ps, h0, w in pss:
            nc.vector.tensor_add(out=ypre[:, h0:h0 + w],
                                 in0=comb_nat[:, h0:h0 + w], in1=ps[:, :w])
        y = tl.tile([128, H], F32)
        ln_apply(y, [(ypre, 0, H)], mlplnw_b, mlplnb_b, gelu=False)
        yT = tl.tile([128, CH, CB], F32R)
        transpose6(y, yT)

        # heads
        mzW_sb = tw.tile([128, CH, FRAG], F32R)
        nc.sync.dma_start(out=mzW_sb[:],
                          in_=T["mzW"].ap().rearrange("(k p) m -> p k m", p=128))
        mzb_sb = tw.tile([1, FRAG], F32R)
        nc.sync.dma_start(out=mzb_sb[:], in_=T["mzb"].ap())
        probW_sb = tw.tile([128, CH, FRAG], F32R)
        nc.sync.dma_start(out=probW_sb[:],
                          in_=T["probW"].ap().rearrange("(k p) m -> p k m", p=128))
        probb_sb = tw.tile([1, FRAG], F32R)
        nc.sync.dma_start(out=probb_sb[:], in_=T["probb"].ap())

        mzs = tl.tile([128, FRAG], F32)
        pss = nat_matmul(lambda k: yT[:, k, :], mzW_sb, mzb_sb, FRAG, CH, "mz")
        for ps, h0, w in pss:
            nc.scalar.activation(out=mzs[:, h0:h0 + w], in_=ps[:, :w],
                                 func=ACTF.Relu)
        nc.vector.tensor_scalar_min(out=mzs[:], in0=mzs[:], scalar1=MZ_MAX)

        pe = tl.tile([128, FRAG], F32)
        pss = nat_matmul(lambda k: yT[:, k, :], probW_sb, probb_sb, FRAG, CH,
                         "pr")
        for ps, h0, w in pss:
            nc.vector.tensor_copy(out=pe[:, h0:h0 + w], in_=ps[:, :w])
        pmx = tl.tile([128, 1], F32)
        nc.vector.tensor_reduce(out=pmx[:], in_=pe[:], axis=AX.X, op=ALU.max)
        nc.scalar.mul(out=pmx[:], in_=pmx[:], mul=-1.0)
        sexp = tl.tile([128, 1], F32)
        nc.scalar.activation(out=pe[:], in_=pe[:], func=ACTF.Exp, bias=pmx[:],
                             accum_out=sexp[:])
        nc.vector.reciprocal(out=sexp[:], in_=sexp[:])
        probs = tl.tile([128, FRAG], F32)
        nc.vector.tensor_scalar_mul(out=probs[:], in0=pe[:], scalar1=sexp[:])
        mask = tl.tile([128, FRAG], F32)
        nc.vector.tensor_scalar(out=mask[:], in0=probs[:], scalar1=PROB_THR,
                                scalar2=None, op0=ALU.is_gt)
        nc.vector.tensor_mul(out=probs[:], in0=probs[:], in1=mask[:])
        den = tl.tile([128, 1], F32)
        nc.vector.tensor_reduce(out=den[:], in_=probs[:], axis=AX.X,
                                op=ALU.add)
        nc.vector.tensor_scalar_add(out=den[:], in0=den[:], scalar1=1e-10)
        nc.vector.reciprocal(out=den[:], in_=den[:])
        nc.vector.tensor_scalar_mul(out=probs[:], in0=probs[:], scalar1=den[:])
        nc.vector.tensor_mul(out=mzs[:], in0=mzs[:], in1=mask[:])

        for f0 in range(0, FRAG, 128):
            nc.sync.dma_start(out=out.ap()[:, f0:f0 + 128, 0],
                              in_=mzs[:, f0:f0 + 128])
            nc.sync.dma_start(out=out.ap()[:, f0:f0 + 128, 1],
                              in_=probs[:, f0:f0 + 128])

    state.release()
    consts.release()


# --------------------------------------------------------------------------
# host side: sharding, exec, unshard
# --------------------------------------------------------------------------

def _host_prep(token_embeddings, supplementary_data, eigenvecs, eigvals,
               edge_index, params):
    p = params
    atom_ids = np.arange(S) % NA
    bond_ids = np.arange(S) % NB
    C = np.zeros((S, 20), np.float32)
    C[np.arange(S), atom_ids] = 1.0
    C[np.arange(S), NA + bond_ids] += 1.0

    evT = np.ascontiguousarray(np.asarray(eigenvecs, np.float32).T)  # [8, N]
    lamN = np.repeat(np.asarray(eigvals, np.float32), NA, axis=0).T  # [8, N]
    phiW1bd = np.zeros((KEIG, 128), np.float32)
    phiW2bd = np.zeros((128, 128), np.float32)
    for k4 in range(4):
        phiW1bd[2 * k4:2 * k4 + 2, 32 * k4:32 * k4 + 32] = p["phi_W1"]
        phiW2bd[32 * k4:32 * k4 + 32, 32 * k4:32 * k4 + 32] = p["phi_W2"]

    f32 = lambda a: np.ascontiguousarray(np.asarray(a, np.float32))
    shared = dict(
        aggC=C,
        phiW1bd=phiW1bd, phiW2bd=phiW2bd,
        phib1r=np.tile(f32(p["phi_b1"]), 4),
        phib2r=np.tile(f32(p["phi_b2"]), 4),
        rhoW1=f32(p["rho_W1"]), rhob1=f32(p["rho_b1"]),
        rhoW2=f32(p["rho_W2"]), rhob2=f32(p["rho_b2"]),
        gw1=f32(p["gine_W1"])[:DEPTH], gb1=f32(p["gine_b1"])[:DEPTH],
        gw2=f32(p["gine_W2"])[:DEPTH], gb2=f32(p["gine_b2"])[:DEPTH],
        geps=f32(p["gine_eps"])[:DEPTH],
        ew1=f32(p["eu_W1"])[:DEPTH], eb1=f32(p["eu_b1"])[:DEPTH],
        ew2=f32(p["eu_W2"])[:DEPTH], eb2=f32(p["eu_b2"])[:DEPTH],
        gnw=f32(p["gn_w"])[:DEPTH], gnb=f32(p["gn_b"])[:DEPTH],
        gna=f32(p["gn_a"])[:DEPTH],
        attnW=f32(p["attn_W"]),
        supW=f32(p["sup_W"]), supb=f32(p["sup_b"])[None, :],
        suplnw=f32(p["sup_ln_w"]), suplnb=f32(p["sup_ln_b"]),
        fc1W=f32(p["fc1_W"]), fc1b=f32(p["fc1_b"])[None, :],
        fc2W=f32(p["fc2_W"]), fc2b=f32(p["fc2_b"])[None, :],
        mlplnw=f32(p["mlp_ln_w"]), mlplnb=f32(p["mlp_ln_b"]),
        mzW=f32(p["mz_W"]), mzb=f32(p["mz_b"])[None, :],
        probW=f32(p["prob_W"]), probb=f32(p["prob_b"])[None, :],
        ones1=np.ones((1, 128), np.float32),
    )

    tok_f = np.asarray(token_embeddings, np.float32)
    sup_f = np.ascontiguousarray(np.asarray(supplementary_data, np.float32).T)

    in_maps = []
    for c in range(NCORES):
        m0, m1 = c * CB, (c + 1) * CB
        n0, n1 = m0 * NA, m1 * NA
        v = evT[:, n0:n1]
        lam = lamN[:, n0:n1]
        u4a = np.zeros((KEIG, 2 * NN), np.float32)
        u4b = np.zeros((KEIG, 2 * NN), np.float32)
        for k4 in range(4):
            u4a[2 * k4, :NN] = v[k4]
            u4a[2 * k4, NN:] = -v[k4]
            u4a[2 * k4 + 1, :NN] = lam[k4]
            u4a[2 * k4 + 1, NN:] = lam[k4]
            u4b[2 * k4, :NN] = v[4 + k4]
            u4b[2 * k4, NN:] = -v[4 + k4]
            u4b[2 * k4 + 1, :NN] = lam[4 + k4]
            u4b[2 * k4 + 1, NN:] = lam[4 + k4]
        im = dict(shared)
        im["tok"] = tok_f[m0:m1]
        im["supT"] = np.ascontiguousarray(sup_f[:, m0:m1])
        im["u4a"] = u4a
        im["u4b"] = u4b
        in_maps.append(im)
    return in_maps


def _is_chain(edge_index):
    e = np.arange(E_GLOBAL)
    src = (e // NB) * NA + (e % NB)
    ei = np.asarray(edge_index)
    return ei.shape == (2, E_GLOBAL) and \
        np.array_equal(ei[0], src.astype(ei.dtype)) and \
        np.array_equal(ei[1], (src + 1).astype(ei.dtype))


def _numpy_reference(token_embeddings, supplementary_data, eigenvecs, eigvals,
                     edge_index, params):
    """Slow but general host fallback (only used if edge_index is not the
    contiguous chain the device kernel is specialized for)."""
    from scipy.special import erf
    p = {k: np.asarray(v, np.float64) for k, v in params.items()}
    tok = np.asarray(token_embeddings, np.float64)
    N = B * NA
    atom_ids = np.arange(S) % NA
    bond_ids = np.arange(S) % NB
    Ca = np.zeros((S, NA)); Ca[np.arange(S), atom_ids] = 1
    Cb = np.zeros((S, NB)); Cb[np.arange(S), bond_ids] = 1
    node = np.einsum('bsh,sa->bah', tok, Ca).reshape(N, H)
    edge = np.einsum('bsh,sz->bzh', tok, Cb).reshape(B * NB, H)
    batch = np.repeat(np.arange(B), NA)
    ev = np.asarray(eigvals, np.float64)[batch]
    pair = np.stack([np.asarray(eigenvecs, np.float64), ev], -1)
    flip = np.array([-1.0, 1.0])

    def phi(u):
        h = np.maximum(u @ p['phi_W1'] + p['phi_b1'], 0)
        return h @ p['phi_W2'] + p['phi_b2']

    h = phi(pair) + phi(pair * flip)
    h = h.reshape(N, KEIG * PHI)
    x = node + np.maximum(h @ p['rho_W1'] + p['rho_b1'], 0) @ p['rho_W2'] \
        + p['rho_b2']
    src, dst = np.asarray(edge_index)

    def silu(v):
        return v / (1 + np.exp(-v))

    for l in range(DEPTH):
        agg = np.zeros_like(x)
        np.add.at(agg, dst, np.maximum(x[src] + edge, 0))
        hm = (1 + p['gine_eps'][l]) * x + agg
        x = hm + silu(hm @ p['gine_W1'][l] + p['gine_b1'][l]) \
            @ p['gine_W2'][l] + p['gine_b2'][l]
        e_in = np.concatenate([edge, x[src], x[dst]], 1)
        e = silu(e_in @ p['eu_W1'][l] + p['eu_b1'][l]) @ p['eu_W2'][l] \
            + p['eu_b2'][l]
        mu = e.mean(0)
        ec = e - p['gn_a'][l] * mu
        var = (ec ** 2).mean(0)
        edge = p['gn_w'][l] * ec / np.sqrt(var + 1e-5) + p['gn_b'][l]
    s = (x @ p['attn_W'] + p['attn_b'])[:, 0]
    w = np.exp(s - s.max()); w /= w.sum()
    pooled = np.zeros((B, H))
    np.add.at(pooled, batch, x * w[:, None])
    supp = np.asarray(supplementary_data, np.float64) @ p['sup_W'] + p['sup_b']
    m = supp.mean(-1, keepdims=True)
    v = ((supp - m) ** 2).mean(-1, keepdims=True)
    supp = p['sup_ln_w'] * (supp - m) / np.sqrt(v + 1e-5) + p['sup_ln_b']
    supp = 0.5 * supp * (1 + erf(supp / np.sqrt(2)))
    comb = pooled + supp
    g = comb @ p['fc1_W'] + p['fc1_b']
    g = 0.5 * g * (1 + erf(g / np.sqrt(2)))
    y = comb + g @ p['fc2_W'] + p['fc2_b']
    m = y.mean(-1, keepdims=True)
    v = ((y - m) ** 2).mean(-1, keepdims=True)
    y = p['mlp_ln_w'] * (y - m) / np.sqrt(v + 1e-5) + p['mlp_ln_b']
    mzs = np.minimum(np.maximum(y @ p['mz_W'] + p['mz_b'], 0), MZ_MAX)
    lg = y @ p['prob_W'] + p['prob_b']
    lg -= lg.max(1, keepdims=True)
    probs = np.exp(lg); probs /= probs.sum(1, keepdims=True)
    mask = (probs > PROB_THR).astype(np.float64)
    probs = probs * mask
    probs = probs / (probs.sum(1, keepdims=True) + 1e-10)
    mzs = mzs * mask
    return np.stack([mzs, probs], -1).astype(np.float32)


class _Runner:
    def __init__(self):
        t0 = time.time()
        self.nc = _build_nc()
        self.build_s = time.time() - t0

    def run(self, in_maps):
        from concourse.bass_utils import run_bass_kernel_spmd
        res = run_bass_kernel_spmd(self.nc, in_maps,
                                   core_ids=list(range(NCORES)))
        return res.results


def _get_runner():
    global _RUNNER
    if _RUNNER is None:
        _RUNNER = _Runner()
    return _RUNNER


def kernel(token_embeddings, supplementary_data, eigenvecs, eigvals,
           edge_index, params):
    if not _is_chain(edge_index):
        return _numpy_reference(token_embeddings, supplementary_data,
                                eigenvecs, eigvals, edge_index, params)
    in_maps = _host_prep(token_embeddings, supplementary_data, eigenvecs,
                         eigvals, edge_index, params)
    results = _get_runner().run(in_maps)
    return np.concatenate([r["out"] for r in results], axis=0)
